# revision 10
# baseline (speedup 1.0000x reference)
"""Trainium2 Bass kernel for nn_DGT (gnn_message_passing).

Pipeline per batch (N=1024 nodes, D=128 feat, H=8 heads):
  c = F_c + timestep_emb(t)[:, :, None]
  mhsa: q,k,v = c@W{q,k,v}+b ; per-head softmax(q k^T/sqrt(D)) @ v
  h = ctx @ Wo + bo
  feat = relu(A_t @ (h @ Wg))
  norm = rownorm(feat) (ddof=1) ; out = norm @ norm^T / D

Sharding: data-parallel over batch, 2 batches per core on 8 cores.
Host precomputes: timestep embedding (folded into cT), A^T, scale/bias folds.

Device layout strategy (per batch):
  cT [D=128p, n]  -> qT_h, kT_h [128p, n] (scale d^-1/4, bias folded)
  v natural [m, hd] (unbiased; bv folded into b1 = bo + Wo^T bv)
  scores^T tiles [m_tile=128p, n] = kT_h_slice^T @ qT_h  (fp32r)
  P^T = exp(scores^T)  (ACT, bf16)
  ctx_nat [n_tile, 129] = sum_mt PT_slice^T @ [v_h | ones]  (bf16 matmuls)
     col 128 = softmax denominator -> reciprocal -> per-partition scale
  ctxT = PE-transpose(ctx_nat * r)  -> O-proj (fp32r) -> hT (+b1)
  g[mt] = hT_slice^T @ Wg (fp32)  ;  featT = relu(sum_mt g[mt]^T @ AT[mt]) (fp32r)
  stats via PE transpose of featT; norm_nat = (feat-mu)*rstd; normT via transpose
  corr[nt] = normT_slice^T @ normT / 128  (fp32r)
"""

import math
from contextlib import ExitStack

import numpy as np

import concourse.bass as bass
import concourse.tile as tile
from concourse import bacc
from concourse import mybir
from concourse.masks import make_identity

AF = mybir.ActivationFunctionType
ALU = mybir.AluOpType
F32 = mybir.dt.float32
F32R = mybir.dt.float32r
BF16 = mybir.dt.bfloat16

BS, N, D, H = 16, 1024, 128, 8
MAX_T = 1000
N_CORES = 8
BPC = BS // N_CORES  # batches per core

# dtype knobs
USE_F32R = True   # big matmuls in fp32r (1 cyc/row) instead of fp32 (4 cyc/row)
PV_BF16 = True    # P/v operands of the PV matmul in bf16
TR_BF16 = False   # ctx normalize+transpose chain in bf16 (PE transpose at 1 cyc/row)


DT_R = F32R if USE_F32R else F32


def _mm_cast(ap):
    return ap


def emit(ctx: ExitStack, tc: tile.TileContext, io: dict, bpc: int = BPC,
         reps: int = 1):
    """Emit the whole per-core kernel. io maps name -> DRAM AP."""
    nc = tc.nc
    NT = N // 128  # 8 tiles of 128 along nodes
    dt_p = BF16 if PV_BF16 else F32
    s4 = float(D) ** -0.25

    cT_d, at_d = io["cT"], io["AT"]
    wq_d, wk_d, wv_d, wo_d, wg_d = io["Wq"], io["Wk"], io["Wv"], io["Wo"], io["Wg"]
    bqs_d, bks_d, b1_d = io["bqs"], io["bks"], io["b1"]
    out_d = io["out"]

    const = ctx.enter_context(tc.tile_pool(name="const", bufs=1))
    ct_pool = ctx.enter_context(tc.tile_pool(name="ct", bufs=2))
    qk_pool = ctx.enter_context(tc.tile_pool(name="qk", bufs=2))
    v_pool = ctx.enter_context(tc.tile_pool(name="v", bufs=1))
    pt_pool = ctx.enter_context(tc.tile_pool(name="pt", bufs=2))
    ctxt_pool = ctx.enter_context(tc.tile_pool(name="ctxt", bufs=1))
    ht_pool = ctx.enter_context(tc.tile_pool(name="ht", bufs=2))
    g_pool = ctx.enter_context(tc.tile_pool(name="g", bufs=2))
    ft_pool = ctx.enter_context(tc.tile_pool(name="ft", bufs=1))
    nt_pool = ctx.enter_context(tc.tile_pool(name="nt", bufs=1))
    at_pool = ctx.enter_context(tc.tile_pool(name="at", bufs=1))
    co_pool = ctx.enter_context(tc.tile_pool(name="co", bufs=2))
    st_pool = ctx.enter_context(tc.tile_pool(name="st", bufs=4))

    ps_big = ctx.enter_context(tc.tile_pool(name="ps_big", bufs=2, space="PSUM"))
    ps_ctx = ctx.enter_context(tc.tile_pool(name="ps_ctx", bufs=2, space="PSUM"))
    ps_tr = ctx.enter_context(tc.tile_pool(name="ps_tr", bufs=2, space="PSUM"))

    # ---- constants ----
    wq_sb = const.tile([128, H * D], DT_R, tag="wq")
    nc.sync.dma_start(wq_sb[:], wq_d[:])
    wk_sb = const.tile([128, H * D], DT_R, tag="wk")
    nc.sync.dma_start(wk_sb[:], wk_d[:])
    wv_sb = const.tile([128, H * D], DT_R, tag="wv")
    nc.sync.dma_start(wv_sb[:], wv_d[:])
    wo_sb = const.tile([128, H * D], DT_R, tag="wo")  # head h at cols h*128
    for h in range(H):
        nc.sync.dma_start(wo_sb[:, h * 128:(h + 1) * 128], wo_d[h * 128:(h + 1) * 128, :])
    wg_sb = const.tile([128, 128], F32, tag="wg")
    nc.sync.dma_start(wg_sb[:], wg_d[:])
    bqs_sb = const.tile([128, H], F32, tag="bqs")
    nc.sync.dma_start(bqs_sb[:], bqs_d[:])
    bks_sb = const.tile([128, H], F32, tag="bks")
    nc.sync.dma_start(bks_sb[:], bks_d[:])
    b1_sb = const.tile([128, 1], F32, tag="b1")
    nc.sync.dma_start(b1_sb[:], b1_d[:])
    ident = const.tile([128, 128], F32, tag="ident")
    make_identity(nc, ident[:])
    if TR_BF16:
        ident_b = const.tile([128, 128], BF16, tag="ident_b")
        make_identity(nc, ident_b[:])

    for rep in range(reps):
      for b in range(bpc):
        # ---- load cT, A^T ----
        ct_sb = ct_pool.tile([128, N], DT_R)
        nc.sync.dma_start(ct_sb[:], cT_d[b])
        at_sb = at_pool.tile([128, NT * N], DT_R)
        for mt in range(NT):
            nc.sync.dma_start(at_sb[:, mt * N:(mt + 1) * N], at_d[b, mt])

        # ---- v (all heads, natural [m, hd], unbiased) ----
        # stored with a ones column appended per head: [m, H*(D+1)]
        v_sb = v_pool.tile([128, NT * H * (D + 1)], dt_p)
        v3 = v_sb[:].rearrange("p (j c) -> p j c", c=D + 1)  # j = mt*H + h
        nc.vector.memset(v3[:, :, D:D + 1], 1.0)
        for mt in range(NT):
            ps_v = ps_big.tile([128, H * D], F32, tag="big")
            for q in range(2):
                nc.tensor.matmul(
                    ps_v[:, q * 512:(q + 1) * 512],
                    _mm_cast(ct_sb[:, mt * 128:(mt + 1) * 128]),
                    _mm_cast(wv_sb[:, q * 512:(q + 1) * 512]),
                    start=True, stop=True,
                )
            for h in range(H):
                nc.vector.tensor_copy(
                    v3[:, mt * H + h, 0:D], ps_v[:, h * D:(h + 1) * D]
                )

        ctxt_sb = ctxt_pool.tile([128, H * N], DT_R)  # ctxT, head h at cols h*N

        for h in range(H):
            # ---- qT_h, kT_h [128, N] with scale d^-1/4 and bias folded ----
            ps_q = ps_big.tile([128, N], F32, tag="big")
            for q in range(2):
                nc.tensor.matmul(
                    ps_q[:, q * 512:(q + 1) * 512],
                    _mm_cast(wq_sb[:, h * D:(h + 1) * D]),
                    _mm_cast(ct_sb[:, q * 512:(q + 1) * 512]),
                    start=True, stop=True,
                )
            qt_sb = qk_pool.tile([128, N], DT_R, tag="qt")
            nc.vector.tensor_scalar(
                qt_sb[:], ps_q[:], s4, bqs_sb[:, h:h + 1],
                op0=ALU.mult, op1=ALU.add,
            )
            ps_k = ps_big.tile([128, N], F32, tag="big")
            for q in range(2):
                nc.tensor.matmul(
                    ps_k[:, q * 512:(q + 1) * 512],
                    _mm_cast(wk_sb[:, h * D:(h + 1) * D]),
                    _mm_cast(ct_sb[:, q * 512:(q + 1) * 512]),
                    start=True, stop=True,
                )
            kt_sb = qk_pool.tile([128, N], DT_R, tag="kt")
            nc.vector.tensor_scalar(
                kt_sb[:], ps_k[:], s4, bks_sb[:, h:h + 1],
                op0=ALU.mult, op1=ALU.add,
            )

            # ---- scores^T tiles + exp -> PT ----
            pt_sb = pt_pool.tile([128, NT * N], dt_p)
            for mt in range(NT):
                ps_s = ps_big.tile([128, N], F32, tag="big")
                for q in range(2):
                    nc.tensor.matmul(
                        ps_s[:, q * 512:(q + 1) * 512],
                        _mm_cast(kt_sb[:, mt * 128:(mt + 1) * 128]),
                        _mm_cast(qt_sb[:, q * 512:(q + 1) * 512]),
                        start=True, stop=True,
                    )
                nc.scalar.activation(
                    pt_sb[:, mt * N:(mt + 1) * N], ps_s[:], AF.Exp
                )

            # ---- PV natural + denominator, normalize, transpose ----
            for nt in range(NT):
                ps_c = ps_ctx.tile([128, D + 1], F32, tag="ctx")
                for mt in range(NT):
                    nc.tensor.matmul(
                        ps_c[:],
                        pt_sb[:, mt * N + nt * 128: mt * N + (nt + 1) * 128],
                        v3[:, mt * H + h, :],
                        start=(mt == 0), stop=(mt == NT - 1),
                    )
                r_col = st_pool.tile([128, 1], F32, tag="r_col")
                nc.vector.reciprocal(r_col[:], ps_c[:, D:D + 1])
                dt_tr = BF16 if TR_BF16 else F32
                id_tr = ident_b if TR_BF16 else ident
                ctxn = st_pool.tile([128, 128], dt_tr, tag="ctxn")
                nc.vector.tensor_scalar_mul(ctxn[:], ps_c[:, 0:D], r_col[:])
                ps_t = ps_tr.tile([128, 128], dt_tr, tag="tr")
                nc.tensor.transpose(ps_t[:], ctxn[:], id_tr[:])
                nc.vector.tensor_copy(
                    ctxt_sb[:, h * N + nt * 128: h * N + (nt + 1) * 128], ps_t[:]
                )

        # ---- O projection: hT = sum_h Wo_h^T @ ctxT_h + b1 ----
        ps_h = ps_big.tile([128, N], F32, tag="big")
        for h in range(H):
            for q in range(2):
                nc.tensor.matmul(
                    ps_h[:, q * 512:(q + 1) * 512],
                    _mm_cast(wo_sb[:, h * D:(h + 1) * D]),
                    _mm_cast(ctxt_sb[:, h * N + q * 512: h * N + (q + 1) * 512]),
                    start=(h == 0), stop=(h == H - 1),
                )
        ht_sb = ht_pool.tile([128, N], F32)
        nc.scalar.activation(ht_sb[:], ps_h[:], AF.Identity, bias=b1_sb[:])

        # ---- g[mt] = (hT slice)^T @ Wg  (natural [m, dg], fp32) ----
        ps_g = ps_big.tile([128, N], F32, tag="big")
        for mt in range(NT):
            nc.tensor.matmul(
                ps_g[:, mt * 128:(mt + 1) * 128],
                ht_sb[:, mt * 128:(mt + 1) * 128],
                wg_sb[:],
                start=True, stop=True,
            )
        g_sb = g_pool.tile([128, N], DT_R)
        nc.vector.tensor_copy(g_sb[:], ps_g[:])

        # ---- featT = relu(sum_mt g[mt]^T @ AT[mt]) ----
        ps_f = ps_big.tile([128, N], F32, tag="big")
        for mt in range(NT):
            for q in range(2):
                nc.tensor.matmul(
                    ps_f[:, q * 512:(q + 1) * 512],
                    _mm_cast(g_sb[:, mt * 128:(mt + 1) * 128]),
                    _mm_cast(at_sb[:, mt * N + q * 512: mt * N + (q + 1) * 512]),
                    start=(mt == 0), stop=(mt == NT - 1),
                )
        ft_sb = ft_pool.tile([128, N], F32)
        nc.scalar.activation(ft_sb[:], ps_f[:], AF.Relu)

        # ---- row stats + normalize (per n-tile via PE transposes) ----
        ntm_sb = nt_pool.tile([128, N], DT_R)  # normT
        for nt in range(NT):
            ps_fn = ps_tr.tile([128, 128], F32, tag="tr")
            nc.tensor.transpose(
                ps_fn[:], ft_sb[:, nt * 128:(nt + 1) * 128], ident[:]
            )
            mu_raw = st_pool.tile([128, 1], F32, tag="mu_raw")
            nc.vector.reduce_sum(mu_raw[:], ps_fn[:], axis=mybir.AxisListType.X)
            sq_scr = st_pool.tile([128, 128], F32, tag="sq_scr")
            sq = st_pool.tile([128, 1], F32, tag="sq")
            nc.scalar.activation(sq_scr[:], ps_fn[:], AF.Square, accum_out=sq[:])
            mu = st_pool.tile([128, 1], F32, tag="mu")
            nc.vector.tensor_scalar_mul(mu[:], mu_raw[:], 1.0 / D)
            t128mu2 = st_pool.tile([128, 1], F32, tag="t128mu2")
            nc.vector.tensor_tensor(t128mu2[:], mu[:], mu_raw[:], op=ALU.mult)
            var = st_pool.tile([128, 1], F32, tag="var")
            nc.vector.tensor_scalar(
                var[:], sq[:], t128mu2[:], 1.0 / (D - 1),
                op0=ALU.subtract, op1=ALU.mult,
            )
            std = st_pool.tile([128, 1], F32, tag="std")
            nc.scalar.sqrt(std[:], var[:])
            rstd = st_pool.tile([128, 1], F32, tag="rstd")
            stdp = st_pool.tile([128, 1], F32, tag="stdp")
            nc.vector.tensor_scalar_add(stdp[:], std[:], 1e-8)
            nc.vector.reciprocal(rstd[:], stdp[:])
            nnat = st_pool.tile([128, 128], F32, tag="nnat")
            nc.vector.tensor_scalar(
                nnat[:], ps_fn[:], mu[:], rstd[:],
                op0=ALU.subtract, op1=ALU.mult,
            )
            ps_nt = ps_tr.tile([128, 128], F32, tag="tr")
            nc.tensor.transpose(ps_nt[:], nnat[:], ident[:])
            nc.vector.tensor_copy(ntm_sb[:, nt * 128:(nt + 1) * 128], ps_nt[:])

        # ---- gram: corr[nt] = normT_slice^T @ normT / D ----
        for nt in range(NT):
            ps_o = ps_big.tile([128, N], F32, tag="big")
            for q in range(2):
                nc.tensor.matmul(
                    ps_o[:, q * 512:(q + 1) * 512],
                    _mm_cast(ntm_sb[:, nt * 128:(nt + 1) * 128]),
                    _mm_cast(ntm_sb[:, q * 512:(q + 1) * 512]),
                    start=True, stop=True,
                )
            co_sb = co_pool.tile([128, N], F32)
            nc.vector.tensor_scalar_mul(co_sb[:], ps_o[:], 1.0 / D)
            nc.sync.dma_start(out_d[b, nt * 128:(nt + 1) * 128, :], co_sb[:])


def build_nc(bpc: int = BPC, reps: int = 1):
    nc = bacc.Bacc("TRN2", num_devices=N_CORES)
    NT = N // 128
    DT_R = F32R if USE_F32R else F32
    io = {
        "cT": nc.dram_tensor("cT", [bpc, D, N], DT_R, kind="ExternalInput").ap(),
        "AT": nc.dram_tensor("AT", [bpc, NT, 128, N], DT_R, kind="ExternalInput").ap(),
        "Wq": nc.dram_tensor("Wq", [D, H * D], DT_R, kind="ExternalInput").ap(),
        "Wk": nc.dram_tensor("Wk", [D, H * D], DT_R, kind="ExternalInput").ap(),
        "Wv": nc.dram_tensor("Wv", [D, H * D], DT_R, kind="ExternalInput").ap(),
        "Wo": nc.dram_tensor("Wo", [H * D, D], DT_R, kind="ExternalInput").ap(),
        "Wg": nc.dram_tensor("Wg", [D, D], F32, kind="ExternalInput").ap(),
        "bqs": nc.dram_tensor("bqs", [D, H], F32, kind="ExternalInput").ap(),
        "bks": nc.dram_tensor("bks", [D, H], F32, kind="ExternalInput").ap(),
        "b1": nc.dram_tensor("b1", [D, 1], F32, kind="ExternalInput").ap(),
        "out": nc.dram_tensor("out", [bpc, N, N], F32, kind="ExternalOutput").ap(),
    }
    with tile.TileContext(nc) as tc:
        with ExitStack() as st:
            emit(st, tc, io, bpc, reps)
    nc.compile()
    return nc


def host_prep(A_t, F_c, t, Wq, bq, Wk, bk, Wv, bv, Wo, bo, Wg):
    """Host-side preprocessing -> per-core input maps."""
    A_t = np.asarray(A_t, np.float32)
    F_c = np.asarray(F_c, np.float32)
    t = np.asarray(t)
    Wq, bq = np.asarray(Wq, np.float32), np.asarray(bq, np.float32)
    Wk, bk = np.asarray(Wk, np.float32), np.asarray(bk, np.float32)
    Wv, bv = np.asarray(Wv, np.float32), np.asarray(bv, np.float32)
    Wo, bo = np.asarray(Wo, np.float32), np.asarray(bo, np.float32)
    Wg = np.asarray(Wg, np.float32)

    half = N // 2
    freqs = np.exp(
        np.arange(half, dtype=np.float32) * -(math.log(MAX_T) / (half - 1))
    )
    e = t.astype(np.float32)[:, None] * freqs[None, :]
    temb = np.concatenate([np.sin(e), np.cos(e)], axis=1).astype(np.float32)

    c = F_c + temb[:, :, None]  # [BS, N, D]
    cT = np.ascontiguousarray(c.transpose(0, 2, 1))  # [BS, D, N]
    AT = np.ascontiguousarray(A_t.transpose(0, 2, 1)).reshape(BS, N // 128, 128, N)

    s4 = float(D) ** -0.25
    bqs = np.ascontiguousarray((bq * s4).reshape(H, D).T)  # [D, H]
    bks = np.ascontiguousarray((bk * s4).reshape(H, D).T)
    b1 = (bo + Wo.T @ bv).reshape(D, 1).astype(np.float32)

    shared = {
        "Wq": Wq, "Wk": Wk, "Wv": Wv, "Wo": Wo, "Wg": Wg,
        "bqs": bqs, "bks": bks, "b1": b1,
    }
    in_maps = []
    for core in range(N_CORES):
        sl = slice(core * BPC, (core + 1) * BPC)
        m = dict(shared)
        m["cT"] = np.ascontiguousarray(cT[sl])
        m["AT"] = np.ascontiguousarray(AT[sl])
        in_maps.append(m)
    return in_maps


_CACHED_NC = None


def _execute(in_maps, trace=False, **kw):
    global _CACHED_NC
    from concourse.bass_utils import run_bass_kernel_spmd

    if _CACHED_NC is None:
        _CACHED_NC = build_nc()
    return run_bass_kernel_spmd(
        _CACHED_NC, in_maps, list(range(N_CORES)), trace=trace, **kw
    )


def kernel(**inputs) -> np.ndarray:
    in_maps = host_prep(**inputs)
    res = _execute(in_maps)
    out = np.concatenate([r["out"] for r in res.results], axis=0)
    return out.astype(np.float32)
